# revision 29
# baseline (speedup 1.0000x reference)
# Multi-head attention on 8 Trainium2 NeuronCores — data-parallel over batch.
#
# Problem: x[8,1024,768] @ w_qkv[768,2304] -> q,k,v (12 heads, d=64);
#          softmax(q k^T / 8) v ; proj w_proj[768,768] + b_proj.
# Sharding: one batch element per core (8 cores), no collectives.
#
# Per-core kernel (all matmuls bf16 on PE, f32 accumulation in PSUM):
#   1. x -> SBUF (4 high-priority DMAs), cast bf16 (DVE), PE-transpose -> xT
#      [768,1024] (evicted by ScalarE, which idles until attention starts)
#   2. v = x @ w_qkv[:, 1536:] stored [N, 65]/head (col 64 = ones);
#      then per head-pair hp: k,q tiles of (x @ w_qkv[:, :1536])^T interleaved
#      with the attention of heads 2hp, 2hp+1 so PE (matmul) and ScalarE (exp)
#      stay concurrently busy.
#   3. attention per (head-pair, q-chunk), heads interleaved so score matmuls
#      alternate PE row groups (even head streams from SBUF partitions 0-63,
#      odd from 64-127): scoresT[k,q] on PE (K=64, pairs of k-tiles into a
#      2-bank PSUM tile) -> one [128,1024] exp on ScalarE (amortizes the
#      352-cycle ACT pipeline fill) -> AV accumulation on PE; the ones column
#      yields the softmax denominator free -> reciprocal_approx_fast +
#      gpsimd partition-broadcast + one DVE mul to normalize.
#      (no max-subtraction: scores are ~N(0,1), exp cannot overflow)
#   4. out = outT-major matmul with w_proj, bias added during PSUM eviction
import sys
import types

import numpy as np


def _install_axon_profile_hook():
    # The NTFF profile hook normally lives in antenv.axon_hooks; this image
    # lacks it, so recreate it from the boot helper (needed only for
    # trace=True; harmless otherwise).
    try:
        import antenv.axon_hooks  # noqa: F401
        return
    except ImportError:
        pass
    try:
        import antenv
        from trn_agent_boot.trn_boot import _ntff_profile_via_ctypes

        m = types.ModuleType("antenv.axon_hooks")
        hook = _ntff_profile_via_ctypes("/opt/axon/libaxon_pjrt.so")
        m.get_axon_ntff_profile_hook = lambda: hook
        m.set_axon_ntff_profile_hook = lambda h: None
        antenv.axon_hooks = m
        sys.modules["antenv.axon_hooks"] = m
    except Exception:
        pass


N, C, H, D = 1024, 768, 12, 64
SCALE = D ** -0.5
NT = N // 128        # 8 token tiles
CT = C // 128        # 6 channel tiles
NQC = N // 512       # 2 q-chunks
E = D + 1            # per-head v width with ones column


def build_kernel():
    import concourse.bass as bass  # noqa: F401
    import concourse.mybir as mybir
    from concourse import bacc
    from concourse.tile import TileContext
    from concourse.masks import make_identity
    from contextlib import ExitStack

    F32 = mybir.dt.float32
    BF16 = mybir.dt.bfloat16
    Exp = mybir.ActivationFunctionType.Exp

    nc = bacc.Bacc()
    x_ext = nc.declare_dram_parameter("x", [N, C], F32, isOutput=False)
    wqkv_ext = nc.declare_dram_parameter("w_qkv", [C, 3 * C], F32, isOutput=False)
    wproj_ext = nc.declare_dram_parameter("w_proj", [C, C], F32, isOutput=False)
    bproj_ext = nc.declare_dram_parameter("b_proj", [C], F32, isOutput=False)
    out_ext = nc.declare_dram_parameter("out", [N, C], F32, isOutput=True)

    with TileContext(nc) as tc, ExitStack() as ctx:
        const = ctx.enter_context(tc.tile_pool(name="const", bufs=1))
        persist = ctx.enter_context(tc.tile_pool(name="persist", bufs=1))
        stage = ctx.enter_context(tc.tile_pool(name="stage", bufs=2))
        psum_mm = ctx.enter_context(tc.tile_pool(name="psum_mm", bufs=2, space="PSUM"))
        psum_sT = ctx.enter_context(tc.tile_pool(name="psum_sT", bufs=2, space="PSUM"))
        psum_av = ctx.enter_context(tc.tile_pool(name="psum_av", bufs=2, space="PSUM"))

        ident = const.tile([128, 128], BF16, tag="ident")
        make_identity(nc, ident)
        bf32 = const.tile([1, C], F32, tag="bf32")
        nc.sync.dma_start(out=bf32[:], in_=bproj_ext[None, :])
        b_bcast = const.tile([128, C], F32, tag="b_bcast")
        nc.gpsimd.partition_broadcast(b_bcast[:], bf32[:])

        # HAM-warming filler: lowest-priority standalone weight loads keep the
        # PE activity monitor busy through the sparse transpose stretch so the
        # clock gate opens (1.2 -> 2.4 GHz) before the dense matmul stream.
        # Lowest priority => the scheduler only slots them into true PE idle.
        _prio = tc.cur_priority
        tc.cur_priority = 10_000_000
        for _ in range(160):
            nc.tensor.ldweights(ident[:])
        tc.cur_priority = _prio

        w_bf = [persist.tile([128, 3 * C], BF16, tag=f"wbf{k}", name=f"wbf{k}")
                for k in range(CT)]
        wp_bf = [persist.tile([128, C], BF16, tag=f"wpbf{k}", name=f"wpbf{k}")
                 for k in range(CT)]
        xT = [persist.tile([128, N], BF16, tag=f"xT{c}", name=f"xT{c}")
              for c in range(CT)]
        qkT = [persist.tile([128, N], BF16, tag=f"qkT{m}", name=f"qkT{m}")
               for m in range(2 * CT)]
        v_aug = [persist.tile([128, H * E], BF16, tag=f"vaug{m}", name=f"vaug{m}")
                 for m in range(NT)]
        outT = [persist.tile([128, N], BF16, tag=f"outT{c}", name=f"outT{c}")
                for c in range(CT)]

        # ---- load x in two high-priority DMAs, cast (DVE), PE-transpose;
        #      xT evictions go to ScalarE (idle until attention starts)
        xpool_cm = tc.tile_pool(name="xpool", bufs=1)
        xpool = xpool_cm.__enter__()
        xall = xpool.tile([128, NT * C], F32, tag="xall", name="xall")
        HT = NT // 4
        with tc.high_priority():
            for half in range(4):
                nc.sync.dma_start(
                    out=xall[:, half * HT * C:(half + 1) * HT * C]
                        .rearrange("p (t c) -> p t c", c=C),
                    in_=x_ext.rearrange("(t p) c -> p t c", p=128)
                        [:, half * HT:(half + 1) * HT, :])
        for t in range(NT):
            xbf = stage.tile([128, C], BF16, tag="xbf", name=f"xbf{t}")
            nc.vector.tensor_copy(xbf[:], xall[:, t * C:(t + 1) * C])
            for c in range(CT):
                trp = psum_av.tile([128, 128], BF16, tag="av", name=f"trp{t}_{c}")
                nc.tensor.transpose(trp[:], xbf[:, c * 128:(c + 1) * 128], ident[:])
                nc.scalar.copy(xT[c][:, t * 128:(t + 1) * 128], trp[:])
        xpool_cm.__exit__(None, None, None)
        expp = ctx.enter_context(tc.tile_pool(name="expp", bufs=4))
        rbp = ctx.enter_context(tc.tile_pool(name="rbp", bufs=2))

        # ---- load w_qkv by column blocks (v, k, q order), cast on DVE ----
        wq_blocks = [(1536, 512), (2048, 256), (768, 512), (1280, 256),
                     (0, 512), (512, 256)]
        for bi, (cs, cw) in enumerate(wq_blocks):
            wcb = stage.tile([128, CT * 512], F32, tag="wcb", name=f"wcb{bi}")
            src = wqkv_ext.rearrange("(k p) c -> p k c", p=128)[:, :, cs:cs + cw]
            nc.sync.dma_start(out=wcb[:, :CT * cw].rearrange("p (k c) -> p k c", k=CT),
                              in_=src)
            for k in range(CT):
                nc.vector.tensor_copy(w_bf[k][:, cs:cs + cw],
                                      wcb[:, k * cw:(k + 1) * cw])

        # ---- load w_proj + cast (overlaps everything) ----
        for k in range(CT):
            wpst = stage.tile([128, C], F32, tag="wpst", name=f"wpst{k}")
            nc.sync.dma_start(out=wpst[:], in_=wproj_ext[k * 128:(k + 1) * 128, :])
            nc.vector.tensor_copy(wp_bf[k][:], wpst[:])

        # ---- v = x @ w_qkv[:,1536:] into v_aug (strided per-head, ones col) ----
        for m in range(NT):
            va = v_aug[m].rearrange("p (h e) -> p h e", e=E)
            nc.vector.memset(va[:, :, D:E], 1.0)
            for n, (cs, cw) in enumerate([(1536, 512), (2048, 256)]):
                vps = psum_mm.tile([128, 512], F32, tag="mm", name=f"vps{m}_{n}")
                for kt in range(CT):
                    nc.tensor.matmul(vps[:, :cw],
                                     xT[kt][:, m * 128:(m + 1) * 128],
                                     w_bf[kt][:, cs:cs + cw],
                                     start=(kt == 0), stop=(kt == CT - 1))
                nh = cw // D
                nc.vector.tensor_copy(
                    va[:, n * 8:n * 8 + nh, 0:D],
                    vps[:, :cw].rearrange("p (h e) -> p h e", e=D))

        # ---- per head-pair: produce k,q tiles, then attention of both heads
        NG = NT // 2  # 4 k-tile pair groups

        def qk_tile(m):
            for n in range(NQC):
                qps = psum_mm.tile([128, 512], F32, tag="mm", name=f"qps{m}_{n}")
                for kt in range(CT):
                    nc.tensor.matmul(qps[:],
                                     w_bf[kt][:, m * 128:(m + 1) * 128],
                                     xT[kt][:, n * 512:(n + 1) * 512],
                                     start=(kt == 0), stop=(kt == CT - 1))
                nc.vector.tensor_copy(qkT[m][:, n * 512:(n + 1) * 512], qps[:])

        def attention_pair(hp):
            # Both heads of a pair interleaved: even head streams from SBUF
            # partitions 0-63 (PE row group 0), odd head from 64-127 (row
            # group 1) — alternating row groups lets the PE prefetch each
            # LDWEIGHTS under the previous matmul's stream.
            qt = qkT[hp]
            kt_t = qkT[CT + hp]
            for qc in range(NQC):
                avs, pexps = {}, {0: [], 1: []}
                for par in (0, 1):
                    avs[par] = psum_av.tile([128, 512], F32, tag="av",
                                            name=f"av{hp}_{qc}_{par}")
                for g in range(NG):
                    sTs = {}
                    for par in (0, 1):
                        sTs[par] = psum_sT.tile([128, 1024], F32, tag="sT",
                                                name=f"sT{hp}_{qc}_{g}_{par}")
                    for j in range(2):
                        kc = 2 * g + j
                        for par in (0, 1):
                            ro = par * D
                            nc.tensor.matmul(
                                sTs[par][:, j * 512:(j + 1) * 512],
                                kt_t[ro:ro + D, kc * 128:(kc + 1) * 128],
                                qt[ro:ro + D, qc * 512:(qc + 1) * 512],
                                start=True, stop=True)
                    for par in (0, 1):
                        pexp = expp.tile([128, 1024], BF16, tag="pexp",
                                         name=f"pexp{hp}_{qc}_{g}_{par}")
                        nc.scalar.activation(pexp[:], sTs[par][:], Exp, scale=SCALE)
                        pexps[par].append(pexp)
                    if g >= 1:  # 1-group skew: AV(g-1) after scores(g)
                        for j in range(2):
                            kc = 2 * (g - 1) + j
                            for par in (0, 1):
                                h = 2 * hp + par
                                nc.tensor.matmul(
                                    avs[par][0:E, :],
                                    v_aug[kc].rearrange("p (h e) -> p h e",
                                                        e=E)[:, h, :],
                                    pexps[par][g - 1][:, j * 512:(j + 1) * 512],
                                    start=(kc == 0), stop=False)
                for j in range(2):
                    kc = 2 * (NG - 1) + j
                    for par in (0, 1):
                        h = 2 * hp + par
                        nc.tensor.matmul(
                            avs[par][0:E, :],
                            v_aug[kc].rearrange("p (h e) -> p h e", e=E)[:, h, :],
                            pexps[par][NG - 1][:, j * 512:(j + 1) * 512],
                            start=False, stop=(kc == NT - 1))
                # normalize: outT[d, q] = av[d, q] / av[64, q]
                # (denominator to SBUF first: custom-DVE ops misread PSUM)
                for par in (0, 1):
                    h, ro, av = 2 * hp + par, par * D, avs[par]
                    den = rbp.tile([1, 512], F32, tag="den", name=f"den{h}_{qc}")
                    nc.vector.tensor_copy(den[:], av[D:E, :])
                    recip = rbp.tile([1, 512], F32, tag="recip",
                                     name=f"rcp{h}_{qc}")
                    nc.vector.reciprocal_approx_fast(recip[:], den[:])
                    rb = rbp.tile([64, 512], F32, tag="rb", name=f"rb{h}_{qc}")
                    nc.gpsimd.partition_broadcast(rb[:], recip[:])
                    nc.vector.tensor_mul(
                        outT[hp][ro:ro + D, qc * 512:(qc + 1) * 512],
                        av[0:D, :], rb[:])

        for hp in range(CT):
            qk_tile(CT + hp)   # k tile for this head pair
            qk_tile(hp)        # q tile
            attention_pair(hp)

        # ---- output projection ----
        for m in range(NT):
            ysb = stage.tile([128, C], F32, tag="ysb", name=f"ysb{m}", bufs=4)
            for n, (cs, cw) in enumerate([(0, 512), (512, 256)]):
                pools = [(psum_mm, "mm"), (psum_av, "av"), (psum_sT, "sT")]
                pp, ptag = pools[(2 * m + n) % 3]
                yps = pp.tile([128, 512], F32, tag=ptag, name=f"yps{m}_{n}")
                for kt in range(CT):
                    nc.tensor.matmul(yps[:, :cw],
                                     outT[kt][:, m * 128:(m + 1) * 128],
                                     wp_bf[kt][:, cs:cs + cw],
                                     start=(kt == 0), stop=(kt == CT - 1))
                nc.vector.tensor_add(ysb[:, cs:cs + cw], yps[:, :cw],
                                     b_bcast[:, cs:cs + cw])
            nc.sync.dma_start(out=out_ext[m * 128:(m + 1) * 128, :], in_=ysb[:])

    nc.finalize()
    return nc


_NC_CACHE = None


def kernel(x, w_qkv, w_proj, b_proj, trace=False, trace_kwargs=None):
    global _NC_CACHE
    _install_axon_profile_hook()
    from concourse.bass_utils import run_bass_kernel_spmd

    if _NC_CACHE is None:
        _NC_CACHE = build_kernel()
    nc = _NC_CACHE

    x = np.asarray(x, dtype=np.float32)
    w_qkv = np.ascontiguousarray(np.asarray(w_qkv, dtype=np.float32))
    w_proj = np.ascontiguousarray(np.asarray(w_proj, dtype=np.float32))
    b_proj = np.ascontiguousarray(np.asarray(b_proj, dtype=np.float32))
    B = x.shape[0]
    in_maps = [{
        "x": np.ascontiguousarray(x[i]),
        "w_qkv": w_qkv,
        "w_proj": w_proj,
        "b_proj": b_proj,
    } for i in range(B)]

    kwargs = {}
    if trace:
        kwargs["trace"] = True
        if trace_kwargs:
            kwargs.update(trace_kwargs)
    res = run_bass_kernel_spmd(nc, in_maps, core_ids=list(range(B)), **kwargs)
    out = np.stack([res.results[i]["out"] for i in range(B)]).astype(np.float32)
    if trace:
        return out, res
    return out


# revision 30
# speedup vs baseline: 1.1823x; 1.1823x over previous
# Multi-head attention on 8 Trainium2 NeuronCores — data-parallel over batch.
#
# Problem: x[8,1024,768] @ w_qkv[768,2304] -> q,k,v (12 heads, d=64);
#          softmax(q k^T / 8) v ; proj w_proj[768,768] + b_proj.
# Sharding: one batch element per core (8 cores), no collectives.
#
# Per-core kernel (all matmuls bf16 on PE, f32 accumulation in PSUM):
#   1. x -> SBUF (4 high-priority DMAs), cast bf16 (DVE), PE-transpose -> xT
#      [768,1024] (evicted by ScalarE, which idles until attention starts)
#   2. v = x @ w_qkv[:, 1536:] stored [N, 65]/head (col 64 = ones);
#      then per head-pair hp: k,q tiles of (x @ w_qkv[:, :1536])^T interleaved
#      with the attention of heads 2hp, 2hp+1 so PE (matmul) and ScalarE (exp)
#      stay concurrently busy.
#   3. attention per (head-pair, q-chunk), heads interleaved so score matmuls
#      alternate PE row groups (even head streams from SBUF partitions 0-63,
#      odd from 64-127): scoresT[k,q] on PE (K=64, pairs of k-tiles into a
#      2-bank PSUM tile) -> one [128,1024] exp on ScalarE (amortizes the
#      352-cycle ACT pipeline fill) -> AV accumulation on PE; the ones column
#      yields the softmax denominator free -> reciprocal_approx_fast +
#      gpsimd partition-broadcast + one DVE mul to normalize.
#      (no max-subtraction: scores are ~N(0,1), exp cannot overflow)
#   4. out = outT-major matmul with w_proj, bias added during PSUM eviction
import sys
import types

import numpy as np


def _install_axon_profile_hook():
    # The NTFF profile hook normally lives in antenv.axon_hooks; this image
    # lacks it, so recreate it from the boot helper (needed only for
    # trace=True; harmless otherwise).
    try:
        import antenv.axon_hooks  # noqa: F401
        return
    except ImportError:
        pass
    try:
        import antenv
        from trn_agent_boot.trn_boot import _ntff_profile_via_ctypes

        m = types.ModuleType("antenv.axon_hooks")
        hook = _ntff_profile_via_ctypes("/opt/axon/libaxon_pjrt.so")
        m.get_axon_ntff_profile_hook = lambda: hook
        m.set_axon_ntff_profile_hook = lambda h: None
        antenv.axon_hooks = m
        sys.modules["antenv.axon_hooks"] = m
    except Exception:
        pass


N, C, H, D = 1024, 768, 12, 64
SCALE = D ** -0.5
NT = N // 128        # 8 token tiles
CT = C // 128        # 6 channel tiles
NQC = N // 512       # 2 q-chunks
E = D + 1            # per-head v width with ones column


def build_kernel():
    import concourse.bass as bass  # noqa: F401
    import concourse.mybir as mybir
    from concourse import bacc
    from concourse.tile import TileContext
    from concourse.masks import make_identity
    from contextlib import ExitStack

    F32 = mybir.dt.float32
    BF16 = mybir.dt.bfloat16
    Exp = mybir.ActivationFunctionType.Exp

    nc = bacc.Bacc()
    x_ext = nc.declare_dram_parameter("x", [N, C], F32, isOutput=False)
    wqkv_ext = nc.declare_dram_parameter("w_qkv", [C, 3 * C], F32, isOutput=False)
    wproj_ext = nc.declare_dram_parameter("w_proj", [C, C], F32, isOutput=False)
    bproj_ext = nc.declare_dram_parameter("b_proj", [C], F32, isOutput=False)
    out_ext = nc.declare_dram_parameter("out", [N, C], F32, isOutput=True)

    with TileContext(nc) as tc, ExitStack() as ctx:
        const = ctx.enter_context(tc.tile_pool(name="const", bufs=1))
        persist = ctx.enter_context(tc.tile_pool(name="persist", bufs=1))
        stage = ctx.enter_context(tc.tile_pool(name="stage", bufs=2))
        psum_mm = ctx.enter_context(tc.tile_pool(name="psum_mm", bufs=2, space="PSUM"))
        psum_sT = ctx.enter_context(tc.tile_pool(name="psum_sT", bufs=2, space="PSUM"))
        psum_av = ctx.enter_context(tc.tile_pool(name="psum_av", bufs=2, space="PSUM"))

        ident = const.tile([128, 128], BF16, tag="ident")
        make_identity(nc, ident)
        bf32 = const.tile([1, C], F32, tag="bf32")
        nc.sync.dma_start(out=bf32[:], in_=bproj_ext[None, :])
        b_bcast = const.tile([128, C], F32, tag="b_bcast")
        nc.gpsimd.partition_broadcast(b_bcast[:], bf32[:])

        # HAM-warming filler: lowest-priority standalone weight loads keep the
        # PE activity monitor busy through the sparse transpose stretch so the
        # clock gate opens (1.2 -> 2.4 GHz) before the dense matmul stream.
        # Lowest priority => the scheduler only slots them into true PE idle.
        _prio = tc.cur_priority
        tc.cur_priority = 10_000_000
        for _ in range(64):
            nc.tensor.ldweights(ident[:])
        tc.cur_priority = _prio

        w_bf = [persist.tile([128, 3 * C], BF16, tag=f"wbf{k}", name=f"wbf{k}")
                for k in range(CT)]
        wp_bf = [persist.tile([128, C], BF16, tag=f"wpbf{k}", name=f"wpbf{k}")
                 for k in range(CT)]
        xT = [persist.tile([128, N], BF16, tag=f"xT{c}", name=f"xT{c}")
              for c in range(CT)]
        qkT = [persist.tile([128, N], BF16, tag=f"qkT{m}", name=f"qkT{m}")
               for m in range(2 * CT)]
        v_aug = [persist.tile([128, H * E], BF16, tag=f"vaug{m}", name=f"vaug{m}")
                 for m in range(NT)]
        outT = [persist.tile([128, N], BF16, tag=f"outT{c}", name=f"outT{c}")
                for c in range(CT)]

        # ---- load x in two high-priority DMAs, cast (DVE), PE-transpose;
        #      xT evictions go to ScalarE (idle until attention starts)
        xpool_cm = tc.tile_pool(name="xpool", bufs=1)
        xpool = xpool_cm.__enter__()
        xall = xpool.tile([128, NT * C], F32, tag="xall", name="xall")
        HT = NT // 4
        with tc.high_priority():
            for half in range(4):
                nc.sync.dma_start(
                    out=xall[:, half * HT * C:(half + 1) * HT * C]
                        .rearrange("p (t c) -> p t c", c=C),
                    in_=x_ext.rearrange("(t p) c -> p t c", p=128)
                        [:, half * HT:(half + 1) * HT, :])
        for t in range(NT):
            xbf = stage.tile([128, C], BF16, tag="xbf", name=f"xbf{t}")
            nc.vector.tensor_copy(xbf[:], xall[:, t * C:(t + 1) * C])
            for c in range(CT):
                trp = psum_av.tile([128, 128], BF16, tag="av", name=f"trp{t}_{c}")
                nc.tensor.transpose(trp[:], xbf[:, c * 128:(c + 1) * 128], ident[:])
                nc.scalar.copy(xT[c][:, t * 128:(t + 1) * 128], trp[:])
        xpool_cm.__exit__(None, None, None)
        expp = ctx.enter_context(tc.tile_pool(name="expp", bufs=4))
        rbp = ctx.enter_context(tc.tile_pool(name="rbp", bufs=2))

        # ---- load w_qkv by column blocks (v, k, q order), cast on DVE ----
        wq_blocks = [(1536, 512), (2048, 256), (768, 512), (1280, 256),
                     (0, 512), (512, 256)]
        for bi, (cs, cw) in enumerate(wq_blocks):
            wcb = stage.tile([128, CT * 512], F32, tag="wcb", name=f"wcb{bi}")
            src = wqkv_ext.rearrange("(k p) c -> p k c", p=128)[:, :, cs:cs + cw]
            nc.sync.dma_start(out=wcb[:, :CT * cw].rearrange("p (k c) -> p k c", k=CT),
                              in_=src)
            for k in range(CT):
                nc.vector.tensor_copy(w_bf[k][:, cs:cs + cw],
                                      wcb[:, k * cw:(k + 1) * cw])

        # ---- load w_proj + cast (overlaps everything) ----
        for k in range(CT):
            wpst = stage.tile([128, C], F32, tag="wpst", name=f"wpst{k}")
            nc.sync.dma_start(out=wpst[:], in_=wproj_ext[k * 128:(k + 1) * 128, :])
            nc.vector.tensor_copy(wp_bf[k][:], wpst[:])

        # ---- v = x @ w_qkv[:,1536:] into v_aug (strided per-head, ones col) ----
        for m in range(NT):
            va = v_aug[m].rearrange("p (h e) -> p h e", e=E)
            nc.vector.memset(va[:, :, D:E], 1.0)
            for n, (cs, cw) in enumerate([(1536, 512), (2048, 256)]):
                vps = psum_mm.tile([128, 512], F32, tag="mm", name=f"vps{m}_{n}")
                for kt in range(CT):
                    nc.tensor.matmul(vps[:, :cw],
                                     xT[kt][:, m * 128:(m + 1) * 128],
                                     w_bf[kt][:, cs:cs + cw],
                                     start=(kt == 0), stop=(kt == CT - 1))
                nh = cw // D
                nc.vector.tensor_copy(
                    va[:, n * 8:n * 8 + nh, 0:D],
                    vps[:, :cw].rearrange("p (h e) -> p h e", e=D))

        # ---- per head-pair: produce k,q tiles, then attention of both heads
        NG = NT // 2  # 4 k-tile pair groups

        def qk_tile(m):
            for n in range(NQC):
                qps = psum_mm.tile([128, 512], F32, tag="mm", name=f"qps{m}_{n}")
                for kt in range(CT):
                    nc.tensor.matmul(qps[:],
                                     w_bf[kt][:, m * 128:(m + 1) * 128],
                                     xT[kt][:, n * 512:(n + 1) * 512],
                                     start=(kt == 0), stop=(kt == CT - 1))
                nc.vector.tensor_copy(qkT[m][:, n * 512:(n + 1) * 512], qps[:])

        def attention_pair(hp):
            # Both heads of a pair interleaved: even head streams from SBUF
            # partitions 0-63 (PE row group 0), odd head from 64-127 (row
            # group 1) — alternating row groups lets the PE prefetch each
            # LDWEIGHTS under the previous matmul's stream.
            qt = qkT[hp]
            kt_t = qkT[CT + hp]
            for qc in range(NQC):
                avs, pexps = {}, {0: [], 1: []}
                for par in (0, 1):
                    avs[par] = psum_av.tile([128, 512], F32, tag="av",
                                            name=f"av{hp}_{qc}_{par}")
                for g in range(NG):
                    sTs = {}
                    for par in (0, 1):
                        sTs[par] = psum_sT.tile([128, 1024], F32, tag="sT",
                                                name=f"sT{hp}_{qc}_{g}_{par}")
                    for j in range(2):
                        kc = 2 * g + j
                        for par in (0, 1):
                            ro = par * D
                            nc.tensor.matmul(
                                sTs[par][:, j * 512:(j + 1) * 512],
                                kt_t[ro:ro + D, kc * 128:(kc + 1) * 128],
                                qt[ro:ro + D, qc * 512:(qc + 1) * 512],
                                start=True, stop=True)
                    for par in (0, 1):
                        pexp = expp.tile([128, 1024], BF16, tag="pexp",
                                         name=f"pexp{hp}_{qc}_{g}_{par}")
                        nc.scalar.activation(pexp[:], sTs[par][:], Exp, scale=SCALE)
                        pexps[par].append(pexp)
                    if g >= 1:  # 1-group skew: AV(g-1) after scores(g)
                        for j in range(2):
                            kc = 2 * (g - 1) + j
                            for par in (0, 1):
                                h = 2 * hp + par
                                nc.tensor.matmul(
                                    avs[par][0:E, :],
                                    v_aug[kc].rearrange("p (h e) -> p h e",
                                                        e=E)[:, h, :],
                                    pexps[par][g - 1][:, j * 512:(j + 1) * 512],
                                    start=(kc == 0), stop=False)
                for j in range(2):
                    kc = 2 * (NG - 1) + j
                    for par in (0, 1):
                        h = 2 * hp + par
                        nc.tensor.matmul(
                            avs[par][0:E, :],
                            v_aug[kc].rearrange("p (h e) -> p h e", e=E)[:, h, :],
                            pexps[par][NG - 1][:, j * 512:(j + 1) * 512],
                            start=False, stop=(kc == NT - 1))
                # normalize: outT[d, q] = av[d, q] / av[64, q]
                # (denominator to SBUF first: custom-DVE ops misread PSUM)
                for par in (0, 1):
                    h, ro, av = 2 * hp + par, par * D, avs[par]
                    den = rbp.tile([1, 512], F32, tag="den", name=f"den{h}_{qc}")
                    nc.vector.tensor_copy(den[:], av[D:E, :])
                    recip = rbp.tile([1, 512], F32, tag="recip",
                                     name=f"rcp{h}_{qc}")
                    nc.vector.reciprocal_approx_fast(recip[:], den[:])
                    rb = rbp.tile([64, 512], F32, tag="rb", name=f"rb{h}_{qc}")
                    nc.gpsimd.partition_broadcast(rb[:], recip[:])
                    nc.vector.tensor_mul(
                        outT[hp][ro:ro + D, qc * 512:(qc + 1) * 512],
                        av[0:D, :], rb[:])

        for hp in range(CT):
            qk_tile(CT + hp)   # k tile for this head pair
            qk_tile(hp)        # q tile
            attention_pair(hp)

        # ---- output projection ----
        for m in range(NT):
            ysb = stage.tile([128, C], F32, tag="ysb", name=f"ysb{m}", bufs=4)
            for n, (cs, cw) in enumerate([(0, 512), (512, 256)]):
                pools = [(psum_mm, "mm"), (psum_av, "av"), (psum_sT, "sT")]
                pp, ptag = pools[(2 * m + n) % 3]
                yps = pp.tile([128, 512], F32, tag=ptag, name=f"yps{m}_{n}")
                for kt in range(CT):
                    nc.tensor.matmul(yps[:, :cw],
                                     outT[kt][:, m * 128:(m + 1) * 128],
                                     wp_bf[kt][:, cs:cs + cw],
                                     start=(kt == 0), stop=(kt == CT - 1))
                nc.vector.tensor_add(ysb[:, cs:cs + cw], yps[:, :cw],
                                     b_bcast[:, cs:cs + cw])
            nc.sync.dma_start(out=out_ext[m * 128:(m + 1) * 128, :], in_=ysb[:])

    nc.finalize()
    return nc


_NC_CACHE = None


def kernel(x, w_qkv, w_proj, b_proj, trace=False, trace_kwargs=None):
    global _NC_CACHE
    _install_axon_profile_hook()
    from concourse.bass_utils import run_bass_kernel_spmd

    if _NC_CACHE is None:
        _NC_CACHE = build_kernel()
    nc = _NC_CACHE

    x = np.asarray(x, dtype=np.float32)
    w_qkv = np.ascontiguousarray(np.asarray(w_qkv, dtype=np.float32))
    w_proj = np.ascontiguousarray(np.asarray(w_proj, dtype=np.float32))
    b_proj = np.ascontiguousarray(np.asarray(b_proj, dtype=np.float32))
    B = x.shape[0]
    in_maps = [{
        "x": np.ascontiguousarray(x[i]),
        "w_qkv": w_qkv,
        "w_proj": w_proj,
        "b_proj": b_proj,
    } for i in range(B)]

    kwargs = {}
    if trace:
        kwargs["trace"] = True
        if trace_kwargs:
            kwargs.update(trace_kwargs)
    res = run_bass_kernel_spmd(nc, in_maps, core_ids=list(range(B)), **kwargs)
    out = np.stack([res.results[i]["out"] for i in range(B)]).astype(np.float32)
    if trace:
        return out, res
    return out


# revision 31
# speedup vs baseline: 1.2034x; 1.0178x over previous
# Multi-head attention on 8 Trainium2 NeuronCores — data-parallel over batch.
#
# Problem: x[8,1024,768] @ w_qkv[768,2304] -> q,k,v (12 heads, d=64);
#          softmax(q k^T / 8) v ; proj w_proj[768,768] + b_proj.
# Sharding: one batch element per core (8 cores), no collectives.
#
# Per-core kernel (all matmuls bf16 on PE, f32 accumulation in PSUM):
#   1. x -> SBUF (4 high-priority DMAs), cast bf16 (DVE), PE-transpose -> xT
#      [768,1024] (evicted by ScalarE, which idles until attention starts)
#   2. v = x @ w_qkv[:, 1536:] stored [N, 65]/head (col 64 = ones);
#      then per head-pair hp: k,q tiles of (x @ w_qkv[:, :1536])^T interleaved
#      with the attention of heads 2hp, 2hp+1 so PE (matmul) and ScalarE (exp)
#      stay concurrently busy.
#   3. attention per (head-pair, q-chunk), heads interleaved so score matmuls
#      alternate PE row groups (even head streams from SBUF partitions 0-63,
#      odd from 64-127): scoresT[k,q] on PE (K=64, pairs of k-tiles into a
#      2-bank PSUM tile) -> one [128,1024] exp on ScalarE (amortizes the
#      352-cycle ACT pipeline fill) -> AV accumulation on PE; the ones column
#      yields the softmax denominator free -> reciprocal_approx_fast +
#      gpsimd partition-broadcast + one DVE mul to normalize.
#      (no max-subtraction: scores are ~N(0,1), exp cannot overflow)
#   4. out = outT-major matmul with w_proj, bias added during PSUM eviction
import sys
import types

import numpy as np


def _install_axon_profile_hook():
    # The NTFF profile hook normally lives in antenv.axon_hooks; this image
    # lacks it, so recreate it from the boot helper (needed only for
    # trace=True; harmless otherwise).
    try:
        import antenv.axon_hooks  # noqa: F401
        return
    except ImportError:
        pass
    try:
        import antenv
        from trn_agent_boot.trn_boot import _ntff_profile_via_ctypes

        m = types.ModuleType("antenv.axon_hooks")
        hook = _ntff_profile_via_ctypes("/opt/axon/libaxon_pjrt.so")
        m.get_axon_ntff_profile_hook = lambda: hook
        m.set_axon_ntff_profile_hook = lambda h: None
        antenv.axon_hooks = m
        sys.modules["antenv.axon_hooks"] = m
    except Exception:
        pass


N, C, H, D = 1024, 768, 12, 64
SCALE = D ** -0.5
NT = N // 128        # 8 token tiles
CT = C // 128        # 6 channel tiles
NQC = N // 512       # 2 q-chunks
E = D + 1            # per-head v width with ones column


def build_kernel():
    import concourse.bass as bass  # noqa: F401
    import concourse.mybir as mybir
    from concourse import bacc
    from concourse.tile import TileContext
    from concourse.masks import make_identity
    from contextlib import ExitStack

    F32 = mybir.dt.float32
    BF16 = mybir.dt.bfloat16
    Exp = mybir.ActivationFunctionType.Exp

    nc = bacc.Bacc()
    x_ext = nc.declare_dram_parameter("x", [N, C], F32, isOutput=False)
    wqkv_ext = nc.declare_dram_parameter("w_qkv", [C, 3 * C], F32, isOutput=False)
    wproj_ext = nc.declare_dram_parameter("w_proj", [C, C], F32, isOutput=False)
    bproj_ext = nc.declare_dram_parameter("b_proj", [C], F32, isOutput=False)
    out_ext = nc.declare_dram_parameter("out", [N, C], F32, isOutput=True)

    with TileContext(nc) as tc, ExitStack() as ctx:
        const = ctx.enter_context(tc.tile_pool(name="const", bufs=1))
        persist = ctx.enter_context(tc.tile_pool(name="persist", bufs=1))
        stage = ctx.enter_context(tc.tile_pool(name="stage", bufs=2))
        psum_mm = ctx.enter_context(tc.tile_pool(name="psum_mm", bufs=2, space="PSUM"))
        psum_sT = ctx.enter_context(tc.tile_pool(name="psum_sT", bufs=2, space="PSUM"))
        psum_av = ctx.enter_context(tc.tile_pool(name="psum_av", bufs=2, space="PSUM"))

        ident = const.tile([128, 128], BF16, tag="ident")
        make_identity(nc, ident)
        bf32 = const.tile([1, C], F32, tag="bf32")
        nc.sync.dma_start(out=bf32[:], in_=bproj_ext[None, :])
        b_bcast = const.tile([128, C], F32, tag="b_bcast")
        nc.gpsimd.partition_broadcast(b_bcast[:], bf32[:])

        w_bf = [persist.tile([128, 3 * C], BF16, tag=f"wbf{k}", name=f"wbf{k}")
                for k in range(CT)]
        wp_bf = [persist.tile([128, C], BF16, tag=f"wpbf{k}", name=f"wpbf{k}")
                 for k in range(CT)]
        xT = [persist.tile([128, N], BF16, tag=f"xT{c}", name=f"xT{c}")
              for c in range(CT)]
        qkT = [persist.tile([128, N], BF16, tag=f"qkT{m}", name=f"qkT{m}")
               for m in range(2 * CT)]
        v_aug = [persist.tile([128, H * E], BF16, tag=f"vaug{m}", name=f"vaug{m}")
                 for m in range(NT)]
        outT = [persist.tile([128, N], BF16, tag=f"outT{c}", name=f"outT{c}")
                for c in range(CT)]

        # ---- load x in two high-priority DMAs, cast (DVE), PE-transpose;
        #      xT evictions go to ScalarE (idle until attention starts)
        xpool_cm = tc.tile_pool(name="xpool", bufs=1)
        xpool = xpool_cm.__enter__()
        xall = xpool.tile([128, NT * C], F32, tag="xall", name="xall")
        HT = NT // 4
        with tc.high_priority():
            for half in range(4):
                nc.sync.dma_start(
                    out=xall[:, half * HT * C:(half + 1) * HT * C]
                        .rearrange("p (t c) -> p t c", c=C),
                    in_=x_ext.rearrange("(t p) c -> p t c", p=128)
                        [:, half * HT:(half + 1) * HT, :])
        for t in range(NT):
            xbf = stage.tile([128, C], BF16, tag="xbf", name=f"xbf{t}")
            nc.vector.tensor_copy(xbf[:], xall[:, t * C:(t + 1) * C])
            for c in range(CT):
                trp = psum_av.tile([128, 128], BF16, tag="av", name=f"trp{t}_{c}")
                nc.tensor.transpose(trp[:], xbf[:, c * 128:(c + 1) * 128], ident[:])
                nc.scalar.copy(xT[c][:, t * 128:(t + 1) * 128], trp[:])
        xpool_cm.__exit__(None, None, None)
        expp = ctx.enter_context(tc.tile_pool(name="expp", bufs=4))
        rbp = ctx.enter_context(tc.tile_pool(name="rbp", bufs=2))

        # ---- load w_qkv by column blocks (v, k, q order), cast on DVE ----
        wq_blocks = [(1536, 512), (2048, 256), (768, 512), (1280, 256),
                     (0, 512), (512, 256)]
        for bi, (cs, cw) in enumerate(wq_blocks):
            wcb = stage.tile([128, CT * 512], F32, tag="wcb", name=f"wcb{bi}")
            src = wqkv_ext.rearrange("(k p) c -> p k c", p=128)[:, :, cs:cs + cw]
            nc.sync.dma_start(out=wcb[:, :CT * cw].rearrange("p (k c) -> p k c", k=CT),
                              in_=src)
            for k in range(CT):
                nc.vector.tensor_copy(w_bf[k][:, cs:cs + cw],
                                      wcb[:, k * cw:(k + 1) * cw])

        # ---- load w_proj + cast (overlaps everything) ----
        for k in range(CT):
            wpst = stage.tile([128, C], F32, tag="wpst", name=f"wpst{k}")
            nc.sync.dma_start(out=wpst[:], in_=wproj_ext[k * 128:(k + 1) * 128, :])
            nc.vector.tensor_copy(wp_bf[k][:], wpst[:])

        # ---- v = x @ w_qkv[:,1536:] into v_aug (strided per-head, ones col) ----
        for m in range(NT):
            va = v_aug[m].rearrange("p (h e) -> p h e", e=E)
            nc.vector.memset(va[:, :, D:E], 1.0)
            for n, (cs, cw) in enumerate([(1536, 512), (2048, 256)]):
                vps = psum_mm.tile([128, 512], F32, tag="mm", name=f"vps{m}_{n}")
                for kt in range(CT):
                    nc.tensor.matmul(vps[:, :cw],
                                     xT[kt][:, m * 128:(m + 1) * 128],
                                     w_bf[kt][:, cs:cs + cw],
                                     start=(kt == 0), stop=(kt == CT - 1))
                nh = cw // D
                nc.vector.tensor_copy(
                    va[:, n * 8:n * 8 + nh, 0:D],
                    vps[:, :cw].rearrange("p (h e) -> p h e", e=D))

        # ---- per head-pair: produce k,q tiles, then attention of both heads
        NG = NT // 2  # 4 k-tile pair groups

        def qk_tile(m):
            for n in range(NQC):
                qps = psum_mm.tile([128, 512], F32, tag="mm", name=f"qps{m}_{n}")
                for kt in range(CT):
                    nc.tensor.matmul(qps[:],
                                     w_bf[kt][:, m * 128:(m + 1) * 128],
                                     xT[kt][:, n * 512:(n + 1) * 512],
                                     start=(kt == 0), stop=(kt == CT - 1))
                nc.vector.tensor_copy(qkT[m][:, n * 512:(n + 1) * 512], qps[:])

        def attention_pair(hp):
            # Both heads of a pair interleaved: even head streams from SBUF
            # partitions 0-63 (PE row group 0), odd head from 64-127 (row
            # group 1) — alternating row groups lets the PE prefetch each
            # LDWEIGHTS under the previous matmul's stream.
            qt = qkT[hp]
            kt_t = qkT[CT + hp]
            for qc in range(NQC):
                avs, pexps = {}, {0: [], 1: []}
                for par in (0, 1):
                    avs[par] = psum_av.tile([128, 512], F32, tag="av",
                                            name=f"av{hp}_{qc}_{par}")
                for g in range(NG):
                    sTs = {}
                    for par in (0, 1):
                        sTs[par] = psum_sT.tile([128, 1024], F32, tag="sT",
                                                name=f"sT{hp}_{qc}_{g}_{par}")
                    for j in range(2):
                        kc = 2 * g + j
                        for par in (0, 1):
                            ro = par * D
                            nc.tensor.matmul(
                                sTs[par][:, j * 512:(j + 1) * 512],
                                kt_t[ro:ro + D, kc * 128:(kc + 1) * 128],
                                qt[ro:ro + D, qc * 512:(qc + 1) * 512],
                                start=True, stop=True)
                    for par in (0, 1):
                        pexp = expp.tile([128, 1024], BF16, tag="pexp",
                                         name=f"pexp{hp}_{qc}_{g}_{par}")
                        nc.scalar.activation(pexp[:], sTs[par][:], Exp, scale=SCALE)
                        pexps[par].append(pexp)
                    if g >= 1:  # 1-group skew: AV(g-1) after scores(g)
                        for j in range(2):
                            kc = 2 * (g - 1) + j
                            for par in (0, 1):
                                h = 2 * hp + par
                                nc.tensor.matmul(
                                    avs[par][0:E, :],
                                    v_aug[kc].rearrange("p (h e) -> p h e",
                                                        e=E)[:, h, :],
                                    pexps[par][g - 1][:, j * 512:(j + 1) * 512],
                                    start=(kc == 0), stop=False)
                for j in range(2):
                    kc = 2 * (NG - 1) + j
                    for par in (0, 1):
                        h = 2 * hp + par
                        nc.tensor.matmul(
                            avs[par][0:E, :],
                            v_aug[kc].rearrange("p (h e) -> p h e", e=E)[:, h, :],
                            pexps[par][NG - 1][:, j * 512:(j + 1) * 512],
                            start=False, stop=(kc == NT - 1))
                # normalize: outT[d, q] = av[d, q] / av[64, q]
                # (denominator to SBUF first: custom-DVE ops misread PSUM)
                for par in (0, 1):
                    h, ro, av = 2 * hp + par, par * D, avs[par]
                    den = rbp.tile([1, 512], F32, tag="den", name=f"den{h}_{qc}")
                    nc.vector.tensor_copy(den[:], av[D:E, :])
                    recip = rbp.tile([1, 512], F32, tag="recip",
                                     name=f"rcp{h}_{qc}")
                    nc.vector.reciprocal_approx_fast(recip[:], den[:])
                    rb = rbp.tile([64, 512], F32, tag="rb", name=f"rb{h}_{qc}")
                    nc.gpsimd.partition_broadcast(rb[:], recip[:])
                    nc.vector.tensor_mul(
                        outT[hp][ro:ro + D, qc * 512:(qc + 1) * 512],
                        av[0:D, :], rb[:])

        for hp in range(CT):
            qk_tile(CT + hp)   # k tile for this head pair
            qk_tile(hp)        # q tile
            attention_pair(hp)

        # ---- output projection ----
        for m in range(NT):
            ysb = stage.tile([128, C], F32, tag="ysb", name=f"ysb{m}", bufs=4)
            for n, (cs, cw) in enumerate([(0, 512), (512, 256)]):
                pools = [(psum_mm, "mm"), (psum_av, "av"), (psum_sT, "sT")]
                pp, ptag = pools[(2 * m + n) % 3]
                yps = pp.tile([128, 512], F32, tag=ptag, name=f"yps{m}_{n}")
                for kt in range(CT):
                    nc.tensor.matmul(yps[:, :cw],
                                     outT[kt][:, m * 128:(m + 1) * 128],
                                     wp_bf[kt][:, cs:cs + cw],
                                     start=(kt == 0), stop=(kt == CT - 1))
                nc.vector.tensor_add(ysb[:, cs:cs + cw], yps[:, :cw],
                                     b_bcast[:, cs:cs + cw])
            nc.sync.dma_start(out=out_ext[m * 128:(m + 1) * 128, :], in_=ysb[:])

    nc.finalize()
    return nc


_NC_CACHE = None


def kernel(x, w_qkv, w_proj, b_proj, trace=False, trace_kwargs=None):
    global _NC_CACHE
    _install_axon_profile_hook()
    from concourse.bass_utils import run_bass_kernel_spmd

    if _NC_CACHE is None:
        _NC_CACHE = build_kernel()
    nc = _NC_CACHE

    x = np.asarray(x, dtype=np.float32)
    w_qkv = np.ascontiguousarray(np.asarray(w_qkv, dtype=np.float32))
    w_proj = np.ascontiguousarray(np.asarray(w_proj, dtype=np.float32))
    b_proj = np.ascontiguousarray(np.asarray(b_proj, dtype=np.float32))
    B = x.shape[0]
    in_maps = [{
        "x": np.ascontiguousarray(x[i]),
        "w_qkv": w_qkv,
        "w_proj": w_proj,
        "b_proj": b_proj,
    } for i in range(B)]

    kwargs = {}
    if trace:
        kwargs["trace"] = True
        if trace_kwargs:
            kwargs.update(trace_kwargs)
    res = run_bass_kernel_spmd(nc, in_maps, core_ids=list(range(B)), **kwargs)
    out = np.stack([res.results[i]["out"] for i in range(B)]).astype(np.float32)
    if trace:
        return out, res
    return out
